# revision 1
# baseline (speedup 1.0000x reference)
"""Trainium2 Bass kernel for nn_Attention (LN -> QKV -> softmax attn -> out proj).

Sharding: 8 cores; core c handles batch b=c//4 and heads [4*(c%4), 4*(c%4)+4).
Each core computes two partial output contributions (one per head-pair stack)
of shape [1024, 2048] = (w_out slice).T @ attn_out.T; the host sums the 8
partials per batch, transposes, and adds b_out.

Device pipeline per core (bf16 matmuls, fp32 PSUM accumulate):
  A) LN stats on the PE: Sx = 1^T x, Sxx = 1^T x^2 (x^2 via one ScalarE
     Square) accumulated into PSUM rows; rsqrt(var+eps) via bit-trick seed +
     3 Newton steps on the DVE (no ACT table switches, no slow reciprocal).
  B) QKV on raw x^T with LayerNorm folded in algebraically:
       qkv[t,c] = r[t] * ((x @ W')[t,c] - mu[t]*u[c]) + (beta @ W)[c]
     where W' = gamma-folded (and q-scaled) weights, u = colsum(W').
     The -mu*u term is a rank-1 K=1 matmul accumulated straight into each
     QKV PSUM group. q,k are produced transposed [dh, t] (k stays un-scaled:
     r_k rides the exp's per-partition scale); v comes out natural [t, dh].
  C) Flash-style attention without running max (matches the reference
     exactly: plain exp, denom = sum + eps which is a no-op in fp32): S^T
     tiles via matmul, exp on ScalarE (psum -> sbuf bf16), P@V via matmul
     with a ones-column appended to v so the denominator accumulates in the
     same PSUM tile; per-qp-half denominator reciprocal via DMA-reshape +
     Newton, broadcast back through DRAM.
  D) Output projection split into two independent partials, interleaved into
     the last head's attention loop to overlap the PE.
"""

import contextlib

import numpy as np

import concourse.bass as bass
import concourse.tile as tile
from concourse import bacc, mybir
from concourse import bass_utils

# Problem constants (hardcoded per contract)
B, N, DIM = 2, 2048, 1024
H, DH = 16, 64
INNER = H * DH
LN_EPS = 1e-5
ATTN_EPS = 1e-8
SCALE = DH ** -0.5

# Per-core constants
P = 128
T = N                 # tokens per core (one batch)
TT = T // P           # 16 token tiles of 128
NT4 = T // 512        # 4 token tiles of 512
KD = DIM // P         # 8 contraction tiles
HL = 4                # local heads per core
CQK = 2 * HL * DH     # 512 (q cols + k cols)
CV = HL * DH          # 256 (v cols)
GQK = CQK // P        # 4 col groups of 128
KT = T // P           # 16 key tiles of 128

f32 = mybir.dt.float32
f32r = mybir.dt.float32r
bf16 = mybir.dt.bfloat16
FT = mybir.ActivationFunctionType
ALU = mybir.AluOpType

import ml_dtypes
_BF16 = np.dtype(ml_dtypes.bfloat16)

_CACHE = {}


def _hrows(h):
    """Partition slice for head h within a [128, 2, T] two-stack layout."""
    lo = 64 * (h % 2)
    return slice(lo, lo + 64), h // 2


def _build(has_v0):
    nc = bacc.Bacc("TRN2", target_bir_lowering=False, debug=False)

    xt_d = nc.dram_tensor("xt", [DIM, T], bf16, kind="ExternalInput").ap()
    wqk_d = nc.dram_tensor("wqk", [DIM, CQK], bf16, kind="ExternalInput").ap()
    wv_d = nc.dram_tensor("wv", [DIM, CV], bf16, kind="ExternalInput").ap()
    wout_d = nc.dram_tensor("wout", [2 * P, DIM], f32r, kind="ExternalInput").ap()
    nuqk16_d = nc.dram_tensor("nuqk", [CQK], bf16, kind="ExternalInput").ap()
    nuv_d = nc.dram_tensor("nuv", [CV], bf16, kind="ExternalInput").ap()
    v0qk_d = nc.dram_tensor("v0qk", [CQK], f32, kind="ExternalInput").ap()
    v0v_d = nc.dram_tensor("v0v", [CV], f32, kind="ExternalInput").ap()
    outp0_d = nc.dram_tensor("outp0", [DIM, T], f32, kind="ExternalOutput").ap()
    outp1_d = nc.dram_tensor("outp1", [DIM, T], f32, kind="ExternalOutput").ap()

    with tile.TileContext(nc) as tc, contextlib.ExitStack() as ctx:
        pers = ctx.enter_context(tc.tile_pool(name="pers", bufs=1))
        dram = ctx.enter_context(tc.tile_pool(name="dram", bufs=1, space="DRAM"))

        qkT = pers.tile([P, GQK, T], bf16)          # q/k transposed, heads stacked
        vaug = pers.tile([P, KT, HL, DH + 1], bf16)  # v + ones column
        outT = pers.tile([P, 2, T], f32r)           # attention output (transposed)
        wout_sb = pers.tile([P, 2, DIM], f32r)
        dnm = [pers.tile([1, T], f32, name=f"dnm{i}") for i in range(HL)]
        r_c = pers.tile([P, TT], f32)

        sx_dram = dram.tile([1, T], f32)
        sxx_dram = dram.tile([1, T], f32)
        r_dram = dram.tile([1, T], f32)
        dnm_dram = dram.tile([HL, T], f32)
        rdn_dram = dram.tile([HL, T], f32)

        nc.vector.memset(vaug[:], 1.0)

        # ---------------- Phase A+B: stats + QKV projection ----------------
        with tc.tile_pool(name="pab", bufs=1) as pab, \
             tc.tile_pool(name="pabd", bufs=4) as pabd, \
             tc.tile_pool(name="pgen", bufs=4, space="PSUM") as pgen, \
             tc.tile_pool(name="pgv", bufs=3, space="PSUM") as pgv, \
             tc.tile_pool(name="pgs", bufs=1, space="PSUM") as pgs:

            wqk_sb = pab.tile([P, KD, CQK], bf16)
            nc.sync.dma_start(wqk_sb[:], wqk_d.rearrange("(o p) c -> p o c", p=P))
            wv_sb = pab.tile([P, KD, CV], bf16)
            nc.sync.dma_start(wv_sb[:], wv_d.rearrange("(o p) c -> p o c", p=P))
            uqkr_sb = pab.tile([1, CQK], bf16)
            nc.sync.dma_start(uqkr_sb[:], nuqk16_d[None, :])
            uvr_sb = pab.tile([1, CV], bf16)
            nc.sync.dma_start(uvr_sb[:], nuv_d[None, :])
            nmu_row = pab.tile([1, T], bf16)
            sx_row = pab.tile([1, T], f32)
            sxx_row = pab.tile([1, T], f32)
            ones_col = pab.tile([P, 1], bf16)
            nc.vector.memset(ones_col[:], 1.0)
            if has_v0:
                v0qk_sb = pab.tile([P, GQK], f32)
                nc.sync.dma_start(v0qk_sb[:], v0qk_d.rearrange("(g p) -> p g", p=P))
                v0v_bc = pab.tile([P, CV], f32)
                nc.sync.dma_start(v0v_bc[:], v0v_d[None, :].to_broadcast([P, CV]))

            r_bc = pab.tile([P, T], f32)
            sxc = pab.tile([P, TT], f32)
            sxxc = pab.tile([P, TT], f32)
            mu_cc = pab.tile([P, TT], f32)
            magic = pab.tile([P, TT], mybir.dt.int32)
            nc.vector.memset(magic[:], 0x5F3759DF)
            ex2e = pab.tile([P, TT], f32)
            mu2 = pab.tile([P, TT], f32)
            ve = pab.tile([P, TT], f32)
            y0i = pab.tile([P, TT], mybir.dt.int32)
            t0 = pab.tile([P, TT], f32)

            bones = pab.tile([1, 1], bf16)
            nc.vector.memset(bones[:], 1.0)
            brow = pab.tile([1, 64], bf16)
            nc.vector.memset(brow[:], 1.0)
            warm_ps = pgs.tile([P, 512], f32, tag="st", name="warm0")
            for _ in range(150):
                nc.tensor.matmul(warm_ps[64:65, 0:64], bones[:], brow[:],
                                 start=True, stop=True)

            def load_xt(t4):
                tsl = slice(t4 * 512, (t4 + 1) * 512)
                xt_t = pabd.tile([P, KD, 512], bf16, tag="xt", name=f"xt{t4}")
                for kt in range(KD):
                    nc.sync.dma_start(
                        xt_t[:, kt],
                        xt_d[kt * P:(kt + 1) * P, tsl])
                return xt_t

            xt_tiles = {0: load_xt(0)}

            for t4 in range(NT4):
                tsl = slice(t4 * 512, (t4 + 1) * 512)
                s4 = slice(t4 * 4, t4 * 4 + 4)
                if t4 + 1 < NT4:
                    xt_tiles[t4 + 1] = load_xt(t4 + 1)

                # --- stats on PE: Sx = 1^T x, Sxx = 1^T x^2 (psum rows) ---
                xt_t = xt_tiles.pop(t4)
                xsq = pabd.tile([P, KD, 512], bf16, tag="xsq")
                nc.scalar.activation(xsq[:], xt_t[:], FT.Square)
                ps_st = pgs.tile([P, 512], f32, tag="st", name=f"st{t4}")
                for kt in range(KD):
                    nc.tensor.matmul(ps_st[0:1], ones_col[:], xt_t[:, kt],
                                     start=(kt == 0), stop=(kt == KD - 1))
                for kt in range(KD):
                    nc.tensor.matmul(ps_st[32:33], ones_col[:], xsq[:, kt],
                                     start=(kt == 0), stop=(kt == KD - 1))
                nc.scalar.activation(nmu_row[0:1, tsl], ps_st[0:1], FT.Copy,
                                     scale=-1.0 / DIM)
                nc.scalar.copy(sx_row[0:1, tsl], ps_st[0:1])
                nc.scalar.copy(sxx_row[0:1, tsl], ps_st[32:33])
                # round-trip rows into [128, 4] column layout for rsqrt
                nc.sync.dma_start(sx_dram[0:1, tsl], sx_row[0:1, tsl])
                nc.sync.dma_start(sxx_dram[0:1, tsl], sxx_row[0:1, tsl])
                nc.sync.dma_start(sxc[:, s4],
                                  sx_dram[0, tsl].rearrange("(o p) -> p o", p=P))
                nc.sync.dma_start(sxxc[:, s4],
                                  sxx_dram[0, tsl].rearrange("(o p) -> p o", p=P))

                # --- finalize r = rsqrt(var + eps) in column layout ---
                nc.vector.tensor_scalar(ex2e[:, s4], sxxc[:, s4], 1.0 / DIM,
                                        LN_EPS, ALU.mult, ALU.add)
                nc.vector.tensor_scalar_mul(mu_cc[:, s4], sxc[:, s4], 1.0 / DIM)
                nc.vector.tensor_tensor(mu2[:, s4], mu_cc[:, s4], mu_cc[:, s4],
                                        ALU.mult)
                nc.vector.scalar_tensor_tensor(ve[:, s4], mu2[:, s4], -1.0,
                                               ex2e[:, s4], ALU.mult, ALU.add)
                nc.vector.tensor_scalar(y0i[:, s4],
                                        ve[:, s4].bitcast(mybir.dt.int32), 1,
                                        None, ALU.arith_shift_right)
                nc.vector.tensor_tensor(y0i[:, s4], magic[:, s4], y0i[:, s4],
                                        ALU.subtract)
                y = y0i.bitcast(f32)
                for _ in range(3):
                    nc.vector.tensor_tensor(t0[:, s4], y[:, s4], y[:, s4],
                                            ALU.mult)
                    nc.vector.tensor_tensor(t0[:, s4], t0[:, s4], ve[:, s4],
                                            ALU.mult)
                    nc.vector.tensor_scalar(t0[:, s4], t0[:, s4], -0.5, 1.5,
                                            ALU.mult, ALU.add)
                    nc.vector.tensor_tensor(y[:, s4], y[:, s4], t0[:, s4],
                                            ALU.mult)
                nc.vector.tensor_copy(r_c[:, s4], y[:, s4])

                # stats slab to DRAM, then broadcast across partitions
                nc.sync.dma_start(
                    r_dram[0, tsl].rearrange("(o p) -> p o", p=P), r_c[:, s4])
                nc.sync.dma_start(r_bc[:, tsl],
                                  r_dram[0:1, tsl].to_broadcast([P, 512]))

                # --- QKV matmuls + LN-fold corrections for this slab ---
                pending = []

                def finish_qk(nc=nc, tsl=tsl):
                    g, ps = pending.pop(0)
                    # rank-1 LayerNorm-mean correction: psum += u * (-mu)^T
                    nc.tensor.matmul(ps[:], uqkr_sb[0:1, g * P:(g + 1) * P],
                                     nmu_row[0:1, tsl], start=False, stop=True)
                    # plain copy frees the psum slot; r_q applied in a
                    # deferred in-place pass below (r_k rides the exp scale)
                    if g >= 2:
                        nc.scalar.copy(qkT[:, g, tsl], ps[:])
                    else:
                        nc.vector.tensor_copy(qkT[:, g, tsl], ps[:])
                    if has_v0:
                        nc.vector.tensor_scalar_add(qkT[:, g, tsl],
                                                    qkT[:, g, tsl],
                                                    v0qk_sb[:, g:g + 1])

                for g in (2, 3, 0, 1):      # k groups first: no r_bc dep
                    ps = pgen.tile([P, 512], f32, tag="qk", name=f"qk{t4}_{g}")
                    for kt in range(KD):
                        nc.tensor.matmul(ps[:], wqk_sb[:, kt, g * P:(g + 1) * P],
                                         xt_t[:, kt],
                                         start=(kt == 0), stop=False)
                    pending.append((g, ps))
                    if len(pending) > 1:
                        finish_qk()
                while pending:
                    finish_qk()
                # deferred r_q multiply (in place, both q groups)
                nc.vector.tensor_tensor(
                    qkT[:, 0:2, tsl], qkT[:, 0:2, tsl],
                    r_bc[:, tsl][:, None, :].to_broadcast([P, 2, 512]),
                    ALU.mult)

                for st in range(4):
                    tts = t4 * 4 + st
                    stsl = slice(t4 * 512 + st * P, t4 * 512 + (st + 1) * P)
                    psv = pgv.tile([P, CV], f32, tag="v", name=f"v{t4}_{st}")
                    for kt in range(KD):
                        nc.tensor.matmul(psv[:],
                                         xt_t[:, kt, st * P:(st + 1) * P],
                                         wv_sb[:, kt],
                                         start=(kt == 0), stop=False)
                    nc.tensor.matmul(psv[:], nmu_row[0:1, stsl], uvr_sb[0:1, :],
                                     start=False, stop=True)
                    psv3 = psv.rearrange("p (h d) -> p h d", h=HL)
                    nc.vector.tensor_scalar_mul(vaug[:, tts, :, 0:DH], psv3,
                                                r_c[:, tts:tts + 1])
                    if has_v0:
                        v03 = v0v_bc.rearrange("p (h d) -> p h d", h=HL)
                        nc.vector.tensor_tensor(vaug[:, tts, :, 0:DH],
                                                vaug[:, tts, :, 0:DH],
                                                v03, ALU.add)

        # ---------------- Phase C: attention (+ overlapped out-proj) -------
        with tc.tile_pool(name="pat", bufs=6) as pat, \
             tc.tile_pool(name="pat1", bufs=1) as pat1, \
             tc.tile_pool(name="pdo", bufs=3) as pdo, \
             tc.tile_pool(name="psc", bufs=2, space="PSUM") as psc, \
             tc.tile_pool(name="ppv", bufs=2, space="PSUM") as ppv, \
             tc.tile_pool(name="pop", bufs=2, space="PSUM") as pop:

            dbc = pat1.tile([P, 2, T], f32)
            nc.sync.dma_start(wout_sb[:], wout_d.rearrange("(o p) c -> p o c", p=P))
            wones = pat1.tile([1, 1], bf16)
            nc.vector.memset(wones[:], 1.0)
            wrow = pat1.tile([1, 64], bf16)
            nc.vector.memset(wrow[:], 1.0)

            def keep_warm(n, ps_ap):
                for i in range(n):
                    nc.tensor.matmul(ps_ap, wones[:], wrow[:],
                                     start=True, stop=True)

            def outproj_unit(stk2, od, oc, t4):
                tsl = slice(t4 * 512, (t4 + 1) * 512)
                ps = pop.tile([P, 512], f32, tag="op",
                              name=f"op{stk2}_{oc}_{t4}")
                nc.tensor.matmul(ps[:],
                                 wout_sb[:, stk2, oc * P:(oc + 1) * P],
                                 outT[:, stk2, tsl], start=True, stop=True)
                osb = pdo.tile([P, 512], f32, tag="osb")
                nc.any.tensor_copy(osb[:], ps[:])
                nc.sync.dma_start(od[oc * P:(oc + 1) * P, tsl], osb[:])

            op0_units = [(oc, t4) for oc in range(DIM // P)
                         for t4 in range(NT4)]
            op1_units = [(oc, t4) for t4 in range(NT4)
                         for oc in range(DIM // P)]

            for h in range(HL):
                rows, stk = _hrows(h)
                if h == 0:
                    wsc = psc.tile([P, 2, 512], f32, tag="sc", name="warm_bc")
                    keep_warm(48, wsc[0:1, 0, 0:64])
                for qp in range(2):
                    ps_o = [ppv.tile([DH + 1, 512], f32, tag="pv",
                                     name=f"pv{h}_{qp}_{i}") for i in range(2)]
                    for kt in range(KT):
                        if h == 3 and qp == 0 and op0_units:
                            oc_, t4_ = op0_units.pop(0)
                            outproj_unit(0, outp0_d, oc_, t4_)
                        if h == 3 and qp == 1:
                            if op0_units:
                                oc_, t4_ = op0_units.pop(0)
                                outproj_unit(0, outp0_d, oc_, t4_)
                            if op1_units and op1_units[0][1] < 2 and kt >= 8:
                                oc_, t4_ = op1_units.pop(0)
                                outproj_unit(1, outp1_d, oc_, t4_)
                        ps_s = psc.tile([P, 2, 512], f32, tag="sc",
                                        name=f"sc{h}_{qp}_{kt}")
                        for sub in range(2):
                            qt = qp * 2 + sub
                            nc.tensor.matmul(
                                ps_s[:, sub],
                                qkT[rows, 2 + stk, kt * P:(kt + 1) * P],
                                qkT[rows, stk, qt * 512:(qt + 1) * 512],
                                start=True, stop=True)
                        et = pat.tile([P, 2, 512], bf16, tag="exp",
                                      name=f"et{h}_{qp}_{kt}")
                        nc.scalar.activation(et[:], ps_s[:], FT.Exp,
                                             scale=r_c[:, kt:kt + 1])
                        for sub in range(2):
                            nc.tensor.matmul(ps_o[sub], vaug[:, kt, h, :],
                                             et[:, sub],
                                             start=(kt == 0), stop=(kt == KT - 1))
                    for sub in range(2):
                        qt = qp * 2 + sub
                        qsl = slice(qt * 512, (qt + 1) * 512)
                        nc.vector.tensor_copy(dnm[h][0:1, qsl],
                                              ps_o[sub][DH:DH + 1])
                        nc.vector.tensor_copy(outT[rows, stk, qsl],
                                              ps_o[sub][0:DH])

                    # per-qp-half denominator reciprocal + normalize:
                    # DMA-reshape to [128, 8], bit-trick seed + 3 Newton steps
                    hsl = slice(qp * 1024, (qp + 1) * 1024)
                    nc.sync.dma_start(dnm_dram[h:h + 1, hsl], dnm[h][0:1, hsl])
                    dn2 = pat.tile([P, TT // 2], f32, tag="dn2")
                    nc.sync.dma_start(
                        dn2[:], dnm_dram[h, hsl].rearrange("(p o) -> p o", p=P))
                    rmagic = pat.tile([P, TT // 2], mybir.dt.int32, tag="rmagic")
                    nc.vector.memset(rmagic[:], 0x7EEF362E)
                    yi = pat.tile([P, TT // 2], mybir.dt.int32, tag="yi")
                    nc.vector.tensor_tensor(yi[:], rmagic[:],
                                            dn2[:].bitcast(mybir.dt.int32),
                                            ALU.subtract)
                    yf = yi.bitcast(f32)
                    tn = pat.tile([P, TT // 2], f32, tag="tn")
                    for _ in range(3):
                        nc.vector.tensor_tensor(tn[:], dn2[:], yf[:], ALU.mult)
                        nc.vector.tensor_scalar(tn[:], tn[:], -1.0, 2.0,
                                                ALU.mult, ALU.add)
                        nc.vector.tensor_tensor(yf[:], yf[:], tn[:], ALU.mult)
                    nc.sync.dma_start(
                        rdn_dram[h, hsl].rearrange("(p o) -> p o", p=P), yf[:])
                    nc.sync.dma_start(
                        dbc[rows, stk, hsl],
                        rdn_dram[h:h + 1, hsl].to_broadcast([64, 1024]))
                    nc.vector.tensor_tensor(outT[rows, stk, hsl],
                                            outT[rows, stk, hsl],
                                            dbc[rows, stk, hsl], ALU.mult)
                    # interleave stack-1 out-proj for ready halves during h3
                    if h == 3 and qp == 1 and op1_units:
                        while op1_units and op1_units[0][1] < 2:
                            oc_, t4_ = op1_units.pop(0)
                            outproj_unit(1, outp1_d, oc_, t4_)

            # ------------ Phase D: remaining output projection ----------
            for oc_, t4_ in op0_units:
                outproj_unit(0, outp0_d, oc_, t4_)
            for oc_, t4_ in op1_units:
                outproj_unit(1, outp1_d, oc_, t4_)

    nc.compile()
    return nc


def _prep_inputs(x, ln_gamma, ln_beta, w_qkv, w_out, b_out):
    """Host-side sharding/layout prep. Returns (in_maps, has_v0)."""
    x = np.asarray(x, dtype=np.float32)
    ln_gamma = np.asarray(ln_gamma, dtype=np.float32)
    ln_beta = np.asarray(ln_beta, dtype=np.float32)
    w_qkv = np.asarray(w_qkv, dtype=np.float32)
    w_out = np.asarray(w_out, dtype=np.float32)

    wsc = w_qkv.copy()
    wsc[:, :INNER] *= SCALE                      # fold attn scale into q
    wfold = ln_gamma[:, None] * wsc              # fold LN gamma
    u = wfold.sum(axis=0)                        # [3*INNER]
    v0 = ln_beta @ wsc                           # [3*INNER]
    has_v0 = bool(np.any(v0 != 0.0))

    wq, wk, wv_all = np.split(wfold, 3, axis=1)
    uq, uk, uv_all = np.split(u, 3)
    v0q, v0k, v0v_all = np.split(v0, 3)

    in_maps = []
    for c in range(8):
        b = c // 4
        hs = (c % 4) * HL * DH
        sl = slice(hs, hs + HL * DH)
        xb = x[b]                                           # [2048, 1024]
        wqk_loc = np.concatenate([wq[:, sl], wk[:, sl]], axis=1)  # [1024, 512]
        in_maps.append({
            "xt": np.ascontiguousarray(xb.T).astype(_BF16),
            "wqk": np.ascontiguousarray(wqk_loc).astype(_BF16),
            "wv": np.ascontiguousarray(wv_all[:, sl]).astype(_BF16),
            "wout": np.ascontiguousarray(w_out[sl, :]),
            "nuqk": np.concatenate([uq[sl], uk[sl]]).astype(_BF16),
            "nuv": uv_all[sl].astype(_BF16),
            "v0qk": np.concatenate([v0q[sl], v0k[sl]]).astype(np.float32),
            "v0v": v0v_all[sl].astype(np.float32),
        })
    return in_maps, has_v0


def run(x, ln_gamma, ln_beta, w_qkv, w_out, b_out, trace=False, trace_kwargs=None):
    in_maps, has_v0 = _prep_inputs(x, ln_gamma, ln_beta, w_qkv, w_out, b_out)
    key = ("nc", has_v0)
    if key not in _CACHE:
        _CACHE[key] = _build(has_v0)
    nc = _CACHE[key]
    kwargs = {}
    if trace:
        kwargs = dict(trace=True, trace_cores=[0],
                      stitch_traces=False, **(trace_kwargs or {}))
    res = bass_utils.run_bass_kernel_spmd(
        nc, in_maps, core_ids=list(range(8)), **kwargs)

    b_out = np.asarray(b_out, dtype=np.float32)
    out = np.zeros((B, N, DIM), dtype=np.float32)
    for b in range(B):
        acc = np.zeros((DIM, T), dtype=np.float32)
        for c in range(4 * b, 4 * b + 4):
            acc += res.results[c]["outp0"]
            acc += res.results[c]["outp1"]
        out[b] = acc.T + b_out
    return out, res


def kernel(x, ln_gamma, ln_beta, w_qkv, w_out, b_out):
    out, _ = run(x, ln_gamma, ln_beta, w_qkv, w_out, b_out, trace=False)
    return out



# revision 5
# speedup vs baseline: 1.0030x; 1.0030x over previous
"""Trainium2 Bass kernel for nn_Attention (LN -> QKV -> softmax attn -> out proj).

Sharding: 8 cores; core c handles batch b=c//4 and heads [4*(c%4), 4*(c%4)+4).
Each core computes two partial output contributions (one per head-pair stack)
of shape [1024, 2048] = (w_out slice).T @ attn_out.T; the host sums the 8
partials per batch, transposes, and adds b_out.

Device pipeline per core (bf16 matmuls, fp32 PSUM accumulate):
  A) LN stats on the DVE from a second, natural-layout copy of x:
     Sx via tensor_reduce, Sxx via tensor_tensor_reduce; rsqrt(var+eps) via
     bit-trick seed + 3 Newton steps. Row layouts (nmu, r) reach the other
     engines through one DRAM round-trip + partition-broadcast DMA per half.
  B) QKV on raw x^T with LayerNorm folded algebraically:
       qkv[t,c] = r[t] * ((x @ W')[t,c] - mu[t]*u[c])
     The -mu*u correction rides the PSUM drain as a DVE scalar_tensor_tensor;
     r is folded into BOTH q and k columns (one broadcast multiply), so the
     attention exp needs no per-partition scale. v comes out natural [t, dh].
  C) Flash-style attention without running max (matches the reference
     exactly): S^T tiles via matmul, plain exp on ScalarE (psum -> sbuf bf16),
     P@V via matmul with a ones-column appended to v so the denominator
     accumulates in the same PSUM tile; per-qp-half denominator reciprocal via
     DMA-reshape + Newton, broadcast back through DRAM.
  D) Output projection in bf16, one column-block unit at a time, interleaved
     into the h>=2 attention loops (stack 0) and the h3 tail (stack 1).
"""

import contextlib

import numpy as np

import concourse.bass as bass
import concourse.tile as tile
from concourse import bacc, mybir
from concourse import bass_utils

# Problem constants (hardcoded per contract)
B, N, DIM = 2, 2048, 1024
H, DH = 16, 64
INNER = H * DH
LN_EPS = 1e-5
ATTN_EPS = 1e-8
SCALE = DH ** -0.5

# Per-core constants
P = 128
T = N                 # tokens per core (one batch)
TT = T // P           # 16 token tiles of 128
NT4 = T // 512        # 4 token slabs of 512
KD = DIM // P         # 8 contraction tiles
HL = 4                # local heads per core
CQK = 2 * HL * DH     # 512 (q cols + k cols)
CV = HL * DH          # 256 (v cols)
GQK = CQK // P        # 4 col groups of 128
KT = T // P           # 16 key tiles of 128

f32 = mybir.dt.float32
bf16 = mybir.dt.bfloat16
FT = mybir.ActivationFunctionType
ALU = mybir.AluOpType
AXL = mybir.AxisListType

import ml_dtypes
_BF16 = np.dtype(ml_dtypes.bfloat16)

_CACHE = {}


def _hrows(h):
    """Partition slice for head h within a [128, 2, T] two-stack layout."""
    lo = 64 * (h % 2)
    return slice(lo, lo + 64), h // 2


def _build(has_v0):
    nc = bacc.Bacc("TRN2", target_bir_lowering=False, debug=False)

    xt_d = nc.dram_tensor("xt", [DIM, T], bf16, kind="ExternalInput").ap()
    xn_d = nc.dram_tensor("xn", [T, DIM], bf16, kind="ExternalInput").ap()
    wqk_d = nc.dram_tensor("wqk", [DIM, CQK], bf16, kind="ExternalInput").ap()
    wv_d = nc.dram_tensor("wv", [DIM, CV], bf16, kind="ExternalInput").ap()
    wout_d = nc.dram_tensor("wout", [2 * P, DIM], bf16, kind="ExternalInput").ap()
    uqk_d = nc.dram_tensor("uqk", [CQK], f32, kind="ExternalInput").ap()
    uv_d = nc.dram_tensor("uv", [CV], f32, kind="ExternalInput").ap()
    v0qk_d = nc.dram_tensor("v0qk", [CQK], f32, kind="ExternalInput").ap()
    v0v_d = nc.dram_tensor("v0v", [CV], f32, kind="ExternalInput").ap()
    outp0_d = nc.dram_tensor("outp0", [DIM, T], f32, kind="ExternalOutput").ap()
    outp1_d = nc.dram_tensor("outp1", [DIM, T], f32, kind="ExternalOutput").ap()

    with tile.TileContext(nc) as tc, contextlib.ExitStack() as ctx:
        pers = ctx.enter_context(tc.tile_pool(name="pers", bufs=1))
        dram = ctx.enter_context(tc.tile_pool(name="dram", bufs=1, space="DRAM"))

        qkT = pers.tile([P, GQK, T], bf16)          # q/k transposed, heads stacked
        vaug = pers.tile([P, KT, HL, DH + 1], bf16)  # v + ones column
        outT = pers.tile([P, 2, T], bf16)           # attention output (transposed)
        wout_sb = pers.tile([P, 2, DIM], bf16)
        dnm = pers.tile([1, HL, T], f32)
        dbc = pers.tile([P, 2, T], f32)

        nmu_dram = dram.tile([1, T], f32)
        r16_dram = dram.tile([1, T], bf16)
        dnm_dram = dram.tile([HL, T], f32)
        rdn_dram = dram.tile([HL, T], f32)

        nc.vector.memset(vaug[:], 1.0)

        # ---------------- Phase A+B: stats + QKV projection ----------------
        with tc.tile_pool(name="pab", bufs=1) as pab, \
             tc.tile_pool(name="pabd", bufs=2) as pabd, \
             tc.tile_pool(name="pxn", bufs=2) as pxn, \
             tc.tile_pool(name="pgen", bufs=4, space="PSUM") as pgen, \
             tc.tile_pool(name="pgv", bufs=3, space="PSUM") as pgv:

            # --- input DMAs, priority order ---
            uqk_sb = pab.tile([P, GQK], f32)
            nc.sync.dma_start(uqk_sb[:], uqk_d.rearrange("(g p) -> p g", p=P))
            uv_bc = pab.tile([P, CV], f32)
            nc.sync.dma_start(uv_bc[:], uv_d[None, :].to_broadcast([P, CV]))
            if has_v0:
                v0qk_sb = pab.tile([P, GQK], f32)
                nc.sync.dma_start(v0qk_sb[:], v0qk_d.rearrange("(g p) -> p g", p=P))
                v0v_bc = pab.tile([P, CV], f32)
                nc.sync.dma_start(v0v_bc[:], v0v_d[None, :].to_broadcast([P, CV]))

            wqk_sb = pab.tile([P, KD, CQK], bf16)
            nc.sync.dma_start(wqk_sb[:], wqk_d.rearrange("(o p) c -> p o c", p=P))

            def load_xt(t4):
                tsl = slice(t4 * 512, (t4 + 1) * 512)
                xt_t = pabd.tile([P, KD, 512], bf16, tag="xt", name=f"xt{t4}")
                for kt in range(KD):
                    nc.sync.dma_start(
                        xt_t[:, kt],
                        xt_d[kt * P:(kt + 1) * P, tsl])
                return xt_t

            xt_tiles = {0: load_xt(0)}

            wv_sb = pab.tile([P, KD, CV], bf16)
            nc.sync.dma_start(wv_sb[:], wv_d.rearrange("(o p) c -> p o c", p=P))

            # natural-layout x for DVE stats, two 2MB halves
            xn_t = {}
            for g in range(2):
                xn_t[g] = pxn.tile([P, 2, 4, DIM], bf16, tag="xn", name=f"xn{g}")
                for s in range(2):
                    t4 = g * 2 + s
                    nc.sync.dma_start(
                        xn_t[g][:, s],
                        xn_d[t4 * 512:(t4 + 1) * 512].rearrange(
                            "(o p) c -> p o c", p=P))

            xt_tiles[1] = load_xt(1)

            # --- stats scratch ---
            sx = pab.tile([P, TT], f32)
            sxx = pab.tile([P, TT], f32)
            sq_scr = pab.tile([P, DIM], bf16)
            mu_cc = pab.tile([P, TT], f32)
            nmu_c = pab.tile([P, TT], f32)
            ex2e = pab.tile([P, TT], f32)
            mu2 = pab.tile([P, TT], f32)
            ve = pab.tile([P, TT], f32)
            magic = pab.tile([P, TT], mybir.dt.int32)
            nc.vector.memset(magic[:], 0x5F3759DF)
            y0i = pab.tile([P, TT], mybir.dt.int32)
            t0 = pab.tile([P, TT], f32)
            r_c = pab.tile([P, TT], f32)
            r16_c = pab.tile([P, TT], bf16)
            nmu_bc = pab.tile([P, T], f32)
            r_bc = pab.tile([P, T], bf16)

            # preload the Exp ACT table during startup idle time
            dum = pab.tile([1, 16], f32)
            nc.vector.memset(dum[:], 0.0)
            dume = pab.tile([1, 16], bf16)
            nc.scalar.activation(dume[:], dum[:], FT.Exp)

            # PE warmup during initial DMA wait
            bones = pab.tile([1, 1], bf16)
            nc.vector.memset(bones[:], 1.0)
            brow = pab.tile([1, 64], bf16)
            nc.vector.memset(brow[:], 1.0)
            warm_ps = pgen.tile([P, 512], f32, tag="qk", name="warm0")
            for _ in range(24):
                nc.tensor.matmul(warm_ps[64:65, 0:64], bones[:], brow[:],
                                 start=True, stop=True)

            def stats_half(g):
                """DVE stats for token tiles [4g, 4g+4) x 2 slabs -> r, nmu."""
                s8 = slice(g * 8, g * 8 + 8)
                for s in range(2):
                    for o in range(4):
                        tt = g * 8 + s * 4 + o
                        nc.vector.tensor_reduce(
                            sx[:, tt:tt + 1], xn_t[g][:, s, o], AXL.X, ALU.add)
                        nc.vector.scalar_tensor_tensor(
                            sq_scr[:], xn_t[g][:, s, o], 1.0,
                            xn_t[g][:, s, o], ALU.mult, ALU.mult,
                            accum_out=sxx[:, tt:tt + 1])
                nc.vector.tensor_scalar(ex2e[:, s8], sxx[:, s8], 1.0 / DIM,
                                        LN_EPS, ALU.mult, ALU.add)
                nc.vector.tensor_scalar_mul(mu_cc[:, s8], sx[:, s8], 1.0 / DIM)
                nc.vector.tensor_scalar_mul(nmu_c[:, s8], sx[:, s8], -1.0 / DIM)
                nc.vector.tensor_tensor(mu2[:, s8], mu_cc[:, s8], mu_cc[:, s8],
                                        ALU.mult)
                nc.vector.scalar_tensor_tensor(ve[:, s8], mu2[:, s8], -1.0,
                                               ex2e[:, s8], ALU.mult, ALU.add)
                nc.vector.tensor_scalar(y0i[:, s8],
                                        ve[:, s8].bitcast(mybir.dt.int32), 1,
                                        None, ALU.arith_shift_right)
                nc.vector.tensor_tensor(y0i[:, s8], magic[:, s8], y0i[:, s8],
                                        ALU.subtract)
                y = y0i.bitcast(f32)
                for _ in range(3):
                    nc.vector.tensor_tensor(t0[:, s8], y[:, s8], y[:, s8],
                                            ALU.mult)
                    nc.vector.tensor_tensor(t0[:, s8], t0[:, s8], ve[:, s8],
                                            ALU.mult)
                    nc.vector.tensor_scalar(t0[:, s8], t0[:, s8], -0.5, 1.5,
                                            ALU.mult, ALU.add)
                    nc.vector.tensor_tensor(y[:, s8], y[:, s8], t0[:, s8],
                                            ALU.mult)
                nc.vector.tensor_copy(r_c[:, s8], y[:, s8])
                nc.vector.tensor_copy(r16_c[:, s8], y[:, s8])
                # rows to DRAM, then partition-broadcast
                hsl = slice(g * 1024, (g + 1) * 1024)
                nc.sync.dma_start(
                    nmu_dram[0, hsl].rearrange("(o p) -> p o", p=P),
                    nmu_c[:, s8])
                nc.sync.dma_start(
                    r16_dram[0, hsl].rearrange("(o p) -> p o", p=P),
                    r16_c[:, s8])
                nc.sync.dma_start(nmu_bc[:, hsl],
                                  nmu_dram[0:1, hsl].to_broadcast([P, 1024]))
                nc.sync.dma_start(r_bc[:, hsl],
                                  r16_dram[0:1, hsl].to_broadcast([P, 1024]))

            stats_half(0)

            for t4 in range(NT4):
                tsl = slice(t4 * 512, (t4 + 1) * 512)
                if t4 + 1 < NT4:
                    xt_tiles[t4 + 1] = load_xt(t4 + 1)
                if t4 == 1:
                    stats_half(1)
                xt_t = xt_tiles.pop(t4)

                # --- QKV matmuls + fused LN-fold drains for this slab ---
                for g in (2, 3, 0, 1):      # k groups first
                    ps = pgen.tile([P, 512], f32, tag="qk", name=f"qk{t4}_{g}")
                    for kt in range(KD):
                        nc.tensor.matmul(ps[:], wqk_sb[:, kt, g * P:(g + 1) * P],
                                         xt_t[:, kt],
                                         start=(kt == 0), stop=(kt == KD - 1))
                    # drain with rank-1 mean correction: (nmu * u[g]) + psum
                    nc.vector.scalar_tensor_tensor(
                        qkT[:, g, tsl], nmu_bc[:, tsl], uqk_sb[:, g:g + 1],
                        ps[:], ALU.mult, ALU.add)
                # fold r into both q and k columns
                nc.vector.tensor_tensor(
                    qkT[:, :, tsl], qkT[:, :, tsl],
                    r_bc[:, tsl][:, None, :].to_broadcast([P, GQK, 512]),
                    ALU.mult)
                if has_v0:
                    for g in range(GQK):
                        nc.vector.tensor_scalar_add(qkT[:, g, tsl],
                                                    qkT[:, g, tsl],
                                                    v0qk_sb[:, g:g + 1])

                for st in range(4):
                    tts = t4 * 4 + st
                    psv = pgv.tile([P, CV], f32, tag="v", name=f"v{t4}_{st}")
                    for kt in range(KD):
                        nc.tensor.matmul(psv[:],
                                         xt_t[:, kt, st * P:(st + 1) * P],
                                         wv_sb[:, kt],
                                         start=(kt == 0), stop=(kt == KD - 1))
                    psv3 = psv.rearrange("p (h d) -> p h d", h=HL)
                    uv3 = uv_bc.rearrange("p (h d) -> p h d", h=HL)
                    # v = r * (psum + nmu*u)
                    nc.vector.scalar_tensor_tensor(
                        vaug[:, tts, :, 0:DH], uv3, nmu_c[:, tts:tts + 1],
                        psv3, ALU.mult, ALU.add)
                    nc.vector.tensor_scalar_mul(vaug[:, tts, :, 0:DH],
                                                vaug[:, tts, :, 0:DH],
                                                r_c[:, tts:tts + 1])
                    if has_v0:
                        v03 = v0v_bc.rearrange("p (h d) -> p h d", h=HL)
                        nc.vector.tensor_tensor(vaug[:, tts, :, 0:DH],
                                                vaug[:, tts, :, 0:DH],
                                                v03, ALU.add)

            nc.sync.dma_start(wout_sb[:], wout_d.rearrange("(o p) c -> p o c", p=P))

        # ---------------- Phase C: attention (+ overlapped out-proj) -------
        with tc.tile_pool(name="pat", bufs=6) as pat, \
             tc.tile_pool(name="pdo", bufs=3) as pdo, \
             tc.tile_pool(name="psc", bufs=2, space="PSUM") as psc, \
             tc.tile_pool(name="ppv", bufs=2, space="PSUM") as ppv, \
             tc.tile_pool(name="pop", bufs=2, space="PSUM") as pop:

            def outproj_unit(stk2, od, oc, t4, drain_eng="vector"):
                tsl = slice(t4 * 512, (t4 + 1) * 512)
                ps = pop.tile([P, 512], f32, tag="op",
                              name=f"op{stk2}_{oc}_{t4}")
                nc.tensor.matmul(ps[:],
                                 wout_sb[:, stk2, oc * P:(oc + 1) * P],
                                 outT[:, stk2, tsl], start=True, stop=True)
                osb = pdo.tile([P, 512], f32, tag="osb")
                if drain_eng == "vector":
                    nc.vector.tensor_copy(osb[:], ps[:])
                else:
                    nc.scalar.copy(osb[:], ps[:])
                nc.sync.dma_start(od[oc * P:(oc + 1) * P, tsl], osb[:])

            op0_units = [(oc, t4) for oc in range(DIM // P)
                         for t4 in range(NT4)]
            op1_units = [(oc, t4) for t4 in range(NT4)
                         for oc in range(DIM // P)]

            for h in range(HL):
                rows, stk = _hrows(h)
                for qp in range(2):
                    ps_o = [ppv.tile([DH + 1, 512], f32, tag="pv",
                                     name=f"pv{h}_{qp}_{i}") for i in range(2)]
                    for kt in range(KT):
                        if h >= 2 and (kt % 2 == 0) and op0_units:
                            oc_, t4_ = op0_units.pop(0)
                            outproj_unit(0, outp0_d, oc_, t4_)
                        if h == 3 and qp == 1 and kt >= 8 and \
                                op1_units and op1_units[0][1] < 2:
                            oc_, t4_ = op1_units.pop(0)
                            outproj_unit(1, outp1_d, oc_, t4_)
                        ps_s = psc.tile([P, 2, 512], f32, tag="sc",
                                        name=f"sc{h}_{qp}_{kt}")
                        for sub in range(2):
                            qt = qp * 2 + sub
                            nc.tensor.matmul(
                                ps_s[:, sub],
                                qkT[rows, 2 + stk, kt * P:(kt + 1) * P],
                                qkT[rows, stk, qt * 512:(qt + 1) * 512],
                                start=True, stop=True)
                        et = pat.tile([P, 2, 512], bf16, tag="exp",
                                      name=f"et{h}_{qp}_{kt}")
                        nc.scalar.activation(et[:], ps_s[:], FT.Exp)
                        for sub in range(2):
                            nc.tensor.matmul(ps_o[sub], vaug[:, kt, h, :],
                                             et[:, sub],
                                             start=(kt == 0), stop=(kt == KT - 1))
                    for sub in range(2):
                        qt = qp * 2 + sub
                        qsl = slice(qt * 512, (qt + 1) * 512)
                        nc.vector.tensor_copy(dnm[0:1, h, qsl],
                                              ps_o[sub][DH:DH + 1])
                        nc.vector.tensor_copy(outT[rows, stk, qsl],
                                              ps_o[sub][0:DH])

                    # per-qp-half denominator reciprocal + normalize:
                    # DMA-reshape to [128, 8], bit-trick seed + 3 Newton steps
                    hsl = slice(qp * 1024, (qp + 1) * 1024)
                    nc.sync.dma_start(dnm_dram[h:h + 1, hsl], dnm[0:1, h, hsl])
                    dn2 = pat.tile([P, TT // 2], f32, tag="dn2")
                    nc.sync.dma_start(
                        dn2[:], dnm_dram[h, hsl].rearrange("(p o) -> p o", p=P))
                    rmagic = pat.tile([P, TT // 2], mybir.dt.int32, tag="rmagic")
                    nc.vector.memset(rmagic[:], 0x7EEF362E)
                    yi = pat.tile([P, TT // 2], mybir.dt.int32, tag="yi")
                    nc.vector.tensor_tensor(yi[:], rmagic[:],
                                            dn2[:].bitcast(mybir.dt.int32),
                                            ALU.subtract)
                    yf = yi.bitcast(f32)
                    tn = pat.tile([P, TT // 2], f32, tag="tn")
                    for _ in range(3):
                        nc.vector.tensor_tensor(tn[:], dn2[:], yf[:], ALU.mult)
                        nc.vector.tensor_scalar(tn[:], tn[:], -1.0, 2.0,
                                                ALU.mult, ALU.add)
                        nc.vector.tensor_tensor(yf[:], yf[:], tn[:], ALU.mult)
                    nc.sync.dma_start(
                        rdn_dram[h, hsl].rearrange("(p o) -> p o", p=P), yf[:])
                    nc.sync.dma_start(
                        dbc[rows, stk, hsl],
                        rdn_dram[h:h + 1, hsl].to_broadcast([64, 1024]))
                    nc.vector.tensor_tensor(outT[rows, stk, hsl],
                                            outT[rows, stk, hsl],
                                            dbc[rows, stk, hsl], ALU.mult)
                    # stack-1 out-proj for ready halves at the h3 tail
                    if h == 3 and qp == 1:
                        while op1_units and op1_units[0][1] < 2:
                            oc_, t4_ = op1_units.pop(0)
                            outproj_unit(1, outp1_d, oc_, t4_)

            # ------------ Phase D: remaining output projection ----------
            for i, (oc_, t4_) in enumerate(op0_units):
                outproj_unit(0, outp0_d, oc_, t4_,
                             drain_eng="vector" if i % 2 else "scalar")
            for i, (oc_, t4_) in enumerate(op1_units):
                outproj_unit(1, outp1_d, oc_, t4_,
                             drain_eng="vector" if i % 2 else "scalar")

    nc.compile()
    return nc


def _prep_inputs(x, ln_gamma, ln_beta, w_qkv, w_out, b_out):
    """Host-side sharding/layout prep. Returns (in_maps, has_v0)."""
    x = np.asarray(x, dtype=np.float32)
    ln_gamma = np.asarray(ln_gamma, dtype=np.float32)
    ln_beta = np.asarray(ln_beta, dtype=np.float32)
    w_qkv = np.asarray(w_qkv, dtype=np.float32)
    w_out = np.asarray(w_out, dtype=np.float32)

    wsc = w_qkv.copy()
    wsc[:, :INNER] *= SCALE                      # fold attn scale into q
    wfold = ln_gamma[:, None] * wsc              # fold LN gamma
    u = wfold.sum(axis=0)                        # [3*INNER]
    v0 = ln_beta @ wsc                           # [3*INNER]
    has_v0 = bool(np.any(v0 != 0.0))

    wq, wk, wv_all = np.split(wfold, 3, axis=1)
    uq, uk, uv_all = np.split(u, 3)
    v0q, v0k, v0v_all = np.split(v0, 3)

    in_maps = []
    for c in range(8):
        b = c // 4
        hs = (c % 4) * HL * DH
        sl = slice(hs, hs + HL * DH)
        xb = x[b]                                           # [2048, 1024]
        wqk_loc = np.concatenate([wq[:, sl], wk[:, sl]], axis=1)  # [1024, 512]
        in_maps.append({
            "xt": np.ascontiguousarray(xb.T).astype(_BF16),
            "xn": np.ascontiguousarray(xb).astype(_BF16),
            "wqk": np.ascontiguousarray(wqk_loc).astype(_BF16),
            "wv": np.ascontiguousarray(wv_all[:, sl]).astype(_BF16),
            "wout": np.ascontiguousarray(w_out[sl, :]).astype(_BF16),
            "uqk": np.concatenate([uq[sl], uk[sl]]).astype(np.float32),
            "uv": uv_all[sl].astype(np.float32),
            "v0qk": np.concatenate([v0q[sl], v0k[sl]]).astype(np.float32),
            "v0v": v0v_all[sl].astype(np.float32),
        })
    return in_maps, has_v0


def run(x, ln_gamma, ln_beta, w_qkv, w_out, b_out, trace=False, trace_kwargs=None):
    in_maps, has_v0 = _prep_inputs(x, ln_gamma, ln_beta, w_qkv, w_out, b_out)
    key = ("nc", has_v0)
    if key not in _CACHE:
        _CACHE[key] = _build(has_v0)
    nc = _CACHE[key]
    kwargs = {}
    if trace:
        kwargs = dict(trace=True, trace_cores=[0],
                      stitch_traces=False, **(trace_kwargs or {}))
    res = bass_utils.run_bass_kernel_spmd(
        nc, in_maps, core_ids=list(range(8)), **kwargs)

    b_out = np.asarray(b_out, dtype=np.float32)
    out = np.zeros((B, N, DIM), dtype=np.float32)
    for b in range(B):
        acc = np.zeros((DIM, T), dtype=np.float32)
        for c in range(4 * b, 4 * b + 4):
            acc += res.results[c]["outp0"]
            acc += res.results[c]["outp1"]
        out[b] = acc.T + b_out
    return out, res


def kernel(x, ln_gamma, ln_beta, w_qkv, w_out, b_out):
    out, _ = run(x, ln_gamma, ln_beta, w_qkv, w_out, b_out, trace=False)
    return out


# revision 10
# speedup vs baseline: 1.1034x; 1.1000x over previous
"""Trainium2 Bass kernel for nn_Attention (LN -> QKV -> softmax attn -> out proj).

Sharding: 8 cores; core c handles batch b=c//4 and heads [4*(c%4), 4*(c%4)+4).
Each core computes two partial output contributions (one per head-pair stack)
of shape [1024, 2048] = (w_out slice).T @ attn_out.T; the host sums the 8
partials per batch, transposes, and adds b_out.

Device pipeline per core (bf16 matmuls, fp32 PSUM accumulate):
  A) LN stats on the DVE from a second, natural-layout copy of x:
     Sx via tensor_reduce, Sxx via tensor_tensor_reduce; rsqrt(var+eps) via
     bit-trick seed + 3 Newton steps. Row layouts (nmu, r) reach the other
     engines through one DRAM round-trip + partition-broadcast DMA per half.
  B) QKV on raw x^T with LayerNorm folded algebraically:
       qkv[t,c] = r[t] * ((x @ W')[t,c] - mu[t]*u[c])
     The -mu*u correction rides the PSUM drain as a DVE scalar_tensor_tensor;
     r is folded into BOTH q and k columns (one broadcast multiply), so the
     attention exp needs no per-partition scale. v comes out natural [t, dh].
  C) Flash-style attention without running max (matches the reference
     exactly): S^T tiles via matmul, plain exp on ScalarE (psum -> sbuf bf16),
     P@V via matmul with a ones-column appended to v so the denominator
     accumulates in the same PSUM tile; per-qp-half denominator reciprocal via
     DMA-reshape + Newton, broadcast back through DRAM.
  D) Output projection in bf16, one column-block unit at a time, interleaved
     into the h>=2 attention loops (stack 0) and the h3 tail (stack 1).
"""

import contextlib

import numpy as np

import concourse.bass as bass
import concourse.tile as tile
from concourse import bacc, mybir
from concourse import bass_utils

# Problem constants (hardcoded per contract)
B, N, DIM = 2, 2048, 1024
H, DH = 16, 64
INNER = H * DH
LN_EPS = 1e-5
ATTN_EPS = 1e-8
SCALE = DH ** -0.5

# Per-core constants
P = 128
T = N                 # tokens per core (one batch)
TT = T // P           # 16 token tiles of 128
NT4 = T // 512        # 4 token slabs of 512
KD = DIM // P         # 8 contraction tiles
HL = 4                # local heads per core
CQK = 2 * HL * DH     # 512 (q cols + k cols)
CV = HL * DH          # 256 (v cols)
GQK = CQK // P        # 4 col groups of 128
KT = T // P           # 16 key tiles of 128

f32 = mybir.dt.float32
bf16 = mybir.dt.bfloat16
FT = mybir.ActivationFunctionType
ALU = mybir.AluOpType
AXL = mybir.AxisListType

import ml_dtypes
_BF16 = np.dtype(ml_dtypes.bfloat16)

_CACHE = {}


def _hrows(h):
    """Partition slice for head h within a [128, 2, T] two-stack layout."""
    lo = 64 * (h % 2)
    return slice(lo, lo + 64), h // 2


def _build(has_v0):
    nc = bacc.Bacc("TRN2", target_bir_lowering=False, debug=False)

    xt_d = nc.dram_tensor("xt", [DIM, T], bf16, kind="ExternalInput").ap()
    xn_d = nc.dram_tensor("xn", [T, DIM], bf16, kind="ExternalInput").ap()
    wqk_d = nc.dram_tensor("wqk", [DIM, CQK], bf16, kind="ExternalInput").ap()
    wv_d = nc.dram_tensor("wv", [DIM, CV], bf16, kind="ExternalInput").ap()
    wout_d = nc.dram_tensor("wout", [2 * P, DIM], bf16, kind="ExternalInput").ap()
    uqk_d = nc.dram_tensor("uqk", [CQK], f32, kind="ExternalInput").ap()
    uv_d = nc.dram_tensor("uv", [CV], f32, kind="ExternalInput").ap()
    v0qk_d = nc.dram_tensor("v0qk", [CQK], f32, kind="ExternalInput").ap()
    v0v_d = nc.dram_tensor("v0v", [CV], f32, kind="ExternalInput").ap()
    outp0_d = nc.dram_tensor("outp0", [DIM, T], f32, kind="ExternalOutput").ap()
    outp1_d = nc.dram_tensor("outp1", [DIM, T], f32, kind="ExternalOutput").ap()

    with tile.TileContext(nc) as tc, contextlib.ExitStack() as ctx:
        pers = ctx.enter_context(tc.tile_pool(name="pers", bufs=1))
        dram = ctx.enter_context(tc.tile_pool(name="dram", bufs=1, space="DRAM"))

        qkT = pers.tile([P, GQK, T], bf16)          # q/k transposed, heads stacked
        vaug = pers.tile([P, KT, HL, DH + 1], bf16)  # v + ones column
        outT = pers.tile([P, 2, T], bf16)           # attention output (transposed)
        wout_sb = pers.tile([P, 2, DIM], bf16)
        dnm = pers.tile([1, HL, T], f32)
        dbc = pers.tile([P, 2, T], f32)

        nmu_dram = dram.tile([1, T], f32)
        r16_dram = dram.tile([1, T], bf16)
        dnm_dram = dram.tile([HL, T], f32)
        rdn_dram = dram.tile([HL, T], f32)

        nc.vector.memset(vaug[:], 1.0)

        # ---------------- Phase A+B: stats + QKV projection ----------------
        with tc.tile_pool(name="pab", bufs=1) as pab, \
             tc.tile_pool(name="pabd", bufs=2) as pabd, \
             tc.tile_pool(name="pxn", bufs=2) as pxn, \
             tc.tile_pool(name="pgen", bufs=4, space="PSUM") as pgen, \
             tc.tile_pool(name="pgv", bufs=3, space="PSUM") as pgv:

            # --- input DMAs, priority order ---
            uqk_sb = pab.tile([P, GQK], f32)
            nc.sync.dma_start(uqk_sb[:], uqk_d.rearrange("(g p) -> p g", p=P))
            uv_bc = pab.tile([P, CV], f32)
            nc.sync.dma_start(uv_bc[:], uv_d[None, :].to_broadcast([P, CV]))
            if has_v0:
                v0qk_sb = pab.tile([P, GQK], f32)
                nc.sync.dma_start(v0qk_sb[:], v0qk_d.rearrange("(g p) -> p g", p=P))
                v0v_bc = pab.tile([P, CV], f32)
                nc.sync.dma_start(v0v_bc[:], v0v_d[None, :].to_broadcast([P, CV]))

            wqk_sb = pab.tile([P, KD, CQK], bf16)
            nc.sync.dma_start(wqk_sb[:], wqk_d.rearrange("(o p) c -> p o c", p=P))

            def load_xt(t4):
                tsl = slice(t4 * 512, (t4 + 1) * 512)
                xt_t = pabd.tile([P, KD, 512], bf16, tag="xt", name=f"xt{t4}")
                for kt in range(KD):
                    nc.sync.dma_start(
                        xt_t[:, kt],
                        xt_d[kt * P:(kt + 1) * P, tsl])
                return xt_t

            xt_tiles = {0: load_xt(0)}

            wv_sb = pab.tile([P, KD, CV], bf16)
            nc.sync.dma_start(wv_sb[:], wv_d.rearrange("(o p) c -> p o c", p=P))

            # natural-layout x for DVE stats, two 2MB halves
            xn_t = {}
            for g in range(2):
                xn_t[g] = pxn.tile([P, 2, 4, DIM], bf16, tag="xn", name=f"xn{g}")
                for s in range(2):
                    t4 = g * 2 + s
                    nc.sync.dma_start(
                        xn_t[g][:, s],
                        xn_d[t4 * 512:(t4 + 1) * 512].rearrange(
                            "(o p) c -> p o c", p=P))

            xt_tiles[1] = load_xt(1)

            # --- stats scratch ---
            sx = pab.tile([P, TT], f32)
            sxx = pab.tile([P, TT], f32)
            sq_scr = pab.tile([P, DIM], bf16)
            mu_cc = pab.tile([P, TT], f32)
            nmu_c = pab.tile([P, TT], f32)
            ex2e = pab.tile([P, TT], f32)
            mu2 = pab.tile([P, TT], f32)
            ve = pab.tile([P, TT], f32)
            magic = pab.tile([P, TT], mybir.dt.int32)
            nc.vector.memset(magic[:], 0x5F3759DF)
            y0i = pab.tile([P, TT], mybir.dt.int32)
            t0 = pab.tile([P, TT], f32)
            r_c = pab.tile([P, TT], f32)
            r16_c = pab.tile([P, TT], bf16)
            nmu_bc = pab.tile([P, T], f32)
            r_bc = pab.tile([P, T], bf16)

            # preload the Exp ACT table during startup idle time
            dum = pab.tile([1, 16], f32)
            nc.vector.memset(dum[:], 0.0)
            dume = pab.tile([1, 16], bf16)
            nc.scalar.activation(dume[:], dum[:], FT.Exp)

            # PE warmup during initial DMA wait
            bones = pab.tile([1, 1], bf16)
            nc.vector.memset(bones[:], 1.0)
            brow = pab.tile([1, 64], bf16)
            nc.vector.memset(brow[:], 1.0)
            warm_ps = pgen.tile([P, 512], f32, tag="qk", name="warm0")
            for _ in range(24):
                nc.tensor.matmul(warm_ps[64:65, 0:64], bones[:], brow[:],
                                 start=True, stop=True)

            def stats_sq(g):
                """Sxx for half g on the (otherwise idle) ACT engine."""
                for s in range(2):
                    for o in range(4):
                        tt = g * 8 + s * 4 + o
                        nc.scalar.activation(
                            sq_scr[:], xn_t[g][:, s, o], FT.Square,
                            accum_out=sxx[:, tt:tt + 1])

            def stats_fin(g):
                """DVE Sx + rsqrt finalize for half g -> r, nmu + broadcasts."""
                s8 = slice(g * 8, g * 8 + 8)
                for s in range(2):
                    for o in range(4):
                        tt = g * 8 + s * 4 + o
                        nc.vector.tensor_reduce(
                            sx[:, tt:tt + 1], xn_t[g][:, s, o], AXL.X, ALU.add)
                nc.vector.tensor_scalar(ex2e[:, s8], sxx[:, s8], 1.0 / DIM,
                                        LN_EPS, ALU.mult, ALU.add)
                nc.vector.tensor_scalar_mul(mu_cc[:, s8], sx[:, s8], 1.0 / DIM)
                nc.vector.tensor_scalar_mul(nmu_c[:, s8], sx[:, s8], -1.0 / DIM)
                nc.vector.tensor_tensor(mu2[:, s8], mu_cc[:, s8], mu_cc[:, s8],
                                        ALU.mult)
                nc.vector.scalar_tensor_tensor(ve[:, s8], mu2[:, s8], -1.0,
                                               ex2e[:, s8], ALU.mult, ALU.add)
                nc.vector.tensor_scalar(y0i[:, s8],
                                        ve[:, s8].bitcast(mybir.dt.int32), 1,
                                        None, ALU.arith_shift_right)
                nc.vector.tensor_tensor(y0i[:, s8], magic[:, s8], y0i[:, s8],
                                        ALU.subtract)
                y = y0i.bitcast(f32)
                for _ in range(3):
                    nc.vector.tensor_tensor(t0[:, s8], y[:, s8], y[:, s8],
                                            ALU.mult)
                    nc.vector.tensor_tensor(t0[:, s8], t0[:, s8], ve[:, s8],
                                            ALU.mult)
                    nc.vector.tensor_scalar(t0[:, s8], t0[:, s8], -0.5, 1.5,
                                            ALU.mult, ALU.add)
                    nc.vector.tensor_tensor(y[:, s8], y[:, s8], t0[:, s8],
                                            ALU.mult)
                nc.vector.tensor_copy(r_c[:, s8], y[:, s8])
                nc.vector.tensor_copy(r16_c[:, s8], y[:, s8])
                # rows to DRAM, then partition-broadcast
                hsl = slice(g * 1024, (g + 1) * 1024)
                nc.sync.dma_start(
                    nmu_dram[0, hsl].rearrange("(o p) -> p o", p=P),
                    nmu_c[:, s8])
                nc.sync.dma_start(
                    r16_dram[0, hsl].rearrange("(o p) -> p o", p=P),
                    r16_c[:, s8])
                nc.sync.dma_start(nmu_bc[:, hsl],
                                  nmu_dram[0:1, hsl].to_broadcast([P, 1024]))
                nc.sync.dma_start(r_bc[:, hsl],
                                  r16_dram[0:1, hsl].to_broadcast([P, 1024]))

            stats_sq(0)
            stats_sq(1)
            stats_fin(0)

            for t4 in range(NT4):
                tsl = slice(t4 * 512, (t4 + 1) * 512)
                if t4 + 1 < NT4:
                    xt_tiles[t4 + 1] = load_xt(t4 + 1)
                if t4 == 1:
                    stats_fin(1)
                xt_t = xt_tiles.pop(t4)

                # --- QKV matmuls + fused LN-fold drains for this slab ---
                for g in (2, 3, 0, 1):      # k groups first
                    ps = pgen.tile([P, 512], f32, tag="qk", name=f"qk{t4}_{g}")
                    for kt in range(KD):
                        nc.tensor.matmul(ps[:], wqk_sb[:, kt, g * P:(g + 1) * P],
                                         xt_t[:, kt],
                                         start=(kt == 0), stop=(kt == KD - 1))
                    # drain with rank-1 mean correction: (nmu * u[g]) + psum
                    nc.vector.scalar_tensor_tensor(
                        qkT[:, g, tsl], nmu_bc[:, tsl], uqk_sb[:, g:g + 1],
                        ps[:], ALU.mult, ALU.add)
                # fold r into both q and k columns
                nc.vector.tensor_tensor(
                    qkT[:, :, tsl], qkT[:, :, tsl],
                    r_bc[:, tsl][:, None, :].to_broadcast([P, GQK, 512]),
                    ALU.mult)
                if has_v0:
                    for g in range(GQK):
                        nc.vector.tensor_scalar_add(qkT[:, g, tsl],
                                                    qkT[:, g, tsl],
                                                    v0qk_sb[:, g:g + 1])

                for st in range(4):
                    tts = t4 * 4 + st
                    psv = pgv.tile([P, CV], f32, tag="v", name=f"v{t4}_{st}")
                    for kt in range(KD):
                        nc.tensor.matmul(psv[:],
                                         xt_t[:, kt, st * P:(st + 1) * P],
                                         wv_sb[:, kt],
                                         start=(kt == 0), stop=(kt == KD - 1))
                    psv3 = psv.rearrange("p (h d) -> p h d", h=HL)
                    uv3 = uv_bc.rearrange("p (h d) -> p h d", h=HL)
                    # v = r * (psum + nmu*u)
                    nc.vector.scalar_tensor_tensor(
                        vaug[:, tts, :, 0:DH], uv3, nmu_c[:, tts:tts + 1],
                        psv3, ALU.mult, ALU.add)
                    nc.vector.tensor_scalar_mul(vaug[:, tts, :, 0:DH],
                                                vaug[:, tts, :, 0:DH],
                                                r_c[:, tts:tts + 1])
                    if has_v0:
                        v03 = v0v_bc.rearrange("p (h d) -> p h d", h=HL)
                        nc.vector.tensor_tensor(vaug[:, tts, :, 0:DH],
                                                vaug[:, tts, :, 0:DH],
                                                v03, ALU.add)

            nc.sync.dma_start(wout_sb[:], wout_d.rearrange("(o p) c -> p o c", p=P))

        # ---------------- Phase C: attention (+ overlapped out-proj) -------
        with tc.tile_pool(name="pat", bufs=6) as pat, \
             tc.tile_pool(name="pdo", bufs=3) as pdo, \
             tc.tile_pool(name="psc", bufs=2, space="PSUM") as psc, \
             tc.tile_pool(name="ppv", bufs=2, space="PSUM") as ppv, \
             tc.tile_pool(name="pop", bufs=2, space="PSUM") as pop:

            def outproj_unit(stk2, od, oc, t4, drain_eng="vector", pool=None,
                             dpool=None):
                tsl = slice(t4 * 512, (t4 + 1) * 512)
                ps = (pool or pop).tile([P, 512], f32, tag="op",
                                        name=f"op{stk2}_{oc}_{t4}")
                nc.tensor.matmul(ps[:],
                                 wout_sb[:, stk2, oc * P:(oc + 1) * P],
                                 outT[:, stk2, tsl], start=True, stop=True)
                osb = (dpool or pdo).tile([P, 512], f32, tag="osb")
                if drain_eng == "vector":
                    nc.vector.tensor_copy(osb[:], ps[:])
                else:
                    nc.scalar.copy(osb[:], ps[:])
                nc.sync.dma_start(od[oc * P:(oc + 1) * P, tsl], osb[:])

            op0_units = [(oc, t4) for oc in range(DIM // P)
                         for t4 in range(NT4)]
            op1_units = [(oc, t4) for t4 in range(NT4)
                         for oc in range(DIM // P)]

            for h in range(HL):
                rows, stk = _hrows(h)
                for qp in range(2):
                    ps_o = [ppv.tile([DH + 1, 512], f32, tag="pv",
                                     name=f"pv{h}_{qp}_{i}") for i in range(2)]
                    et_prev = None
                    for kt in range(KT):
                        if h >= 2 and (kt % 2 == 0) and op0_units:
                            oc_, t4_ = op0_units.pop(0)
                            outproj_unit(0, outp0_d, oc_, t4_)
                        if h == 3 and qp == 1 and kt >= 8:
                            for _ in range(2):
                                if op1_units and op1_units[0][1] < 2:
                                    oc_, t4_ = op1_units.pop(0)
                                    outproj_unit(1, outp1_d, oc_, t4_)
                        ps_s = psc.tile([P, 2, 512], f32, tag="sc",
                                        name=f"sc{h}_{qp}_{kt}")
                        for sub in range(2):
                            qt = qp * 2 + sub
                            nc.tensor.matmul(
                                ps_s[:, sub],
                                qkT[rows, 2 + stk, kt * P:(kt + 1) * P],
                                qkT[rows, stk, qt * 512:(qt + 1) * 512],
                                start=True, stop=True)
                        et = pat.tile([P, 2, 512], bf16, tag="exp",
                                      name=f"et{h}_{qp}_{kt}")
                        nc.scalar.activation(et[:], ps_s[:], FT.Exp)
                        # PV runs one kt behind so the next score pair is
                        # already queued on the PE when the exp lands
                        if et_prev is not None:
                            for sub in range(2):
                                nc.tensor.matmul(ps_o[sub],
                                                 vaug[:, kt - 1, h, :],
                                                 et_prev[:, sub],
                                                 start=(kt == 1), stop=False)
                        et_prev = et
                    for sub in range(2):
                        nc.tensor.matmul(ps_o[sub], vaug[:, KT - 1, h, :],
                                         et_prev[:, sub],
                                         start=False, stop=True)
                    for sub in range(2):
                        qt = qp * 2 + sub
                        qsl = slice(qt * 512, (qt + 1) * 512)
                        nc.vector.tensor_copy(dnm[0:1, h, qsl],
                                              ps_o[sub][DH:DH + 1])
                        nc.vector.tensor_copy(outT[rows, stk, qsl],
                                              ps_o[sub][0:DH])

                    # per-qp-half denominator reciprocal + normalize:
                    # DMA-reshape to [128, 8], bit-trick seed + 3 Newton steps
                    hsl = slice(qp * 1024, (qp + 1) * 1024)
                    nc.sync.dma_start(dnm_dram[h:h + 1, hsl], dnm[0:1, h, hsl])
                    dn2 = pat.tile([P, TT // 2], f32, tag="dn2")
                    nc.sync.dma_start(
                        dn2[:], dnm_dram[h, hsl].rearrange("(p o) -> p o", p=P))
                    rmagic = pat.tile([P, TT // 2], mybir.dt.int32, tag="rmagic")
                    nc.vector.memset(rmagic[:], 0x7EEF362E)
                    yi = pat.tile([P, TT // 2], mybir.dt.int32, tag="yi")
                    nc.vector.tensor_tensor(yi[:], rmagic[:],
                                            dn2[:].bitcast(mybir.dt.int32),
                                            ALU.subtract)
                    yf = yi.bitcast(f32)
                    tn = pat.tile([P, TT // 2], f32, tag="tn")
                    for _ in range(3):
                        nc.vector.tensor_tensor(tn[:], dn2[:], yf[:], ALU.mult)
                        nc.vector.tensor_scalar(tn[:], tn[:], -1.0, 2.0,
                                                ALU.mult, ALU.add)
                        nc.vector.tensor_tensor(yf[:], yf[:], tn[:], ALU.mult)
                    nc.sync.dma_start(
                        rdn_dram[h, hsl].rearrange("(p o) -> p o", p=P), yf[:])
                    nc.sync.dma_start(
                        dbc[rows, stk, hsl],
                        rdn_dram[h:h + 1, hsl].to_broadcast([64, 1024]))
                    nc.vector.tensor_tensor(outT[rows, stk, hsl],
                                            outT[rows, stk, hsl],
                                            dbc[rows, stk, hsl], ALU.mult)
                    # stack-1 out-proj for ready halves at the h3 tail
                    if h == 3 and qp == 1:
                        while op1_units and op1_units[0][1] < 2:
                            i = len(op1_units)
                            oc_, t4_ = op1_units.pop(0)
                            outproj_unit(1, outp1_d, oc_, t4_,
                                         drain_eng="vector" if i % 2 else "scalar")

        # ------------ Phase D: remaining output projection ----------
        with tc.tile_pool(name="pdo2", bufs=4) as pdo2, \
             tc.tile_pool(name="pop2", bufs=4, space="PSUM") as pop2:
            rem = [(0, outp0_d, oc_, t4_) for oc_, t4_ in op0_units] + \
                  [(1, outp1_d, oc_, t4_) for oc_, t4_ in op1_units]
            for i, (stk2, od, oc_, t4_) in enumerate(rem):
                outproj_unit(stk2, od, oc_, t4_,
                             drain_eng="vector" if i % 2 else "scalar",
                             pool=pop2, dpool=pdo2)

    nc.compile()
    return nc


def _prep_inputs(x, ln_gamma, ln_beta, w_qkv, w_out, b_out):
    """Host-side sharding/layout prep. Returns (in_maps, has_v0)."""
    x = np.asarray(x, dtype=np.float32)
    ln_gamma = np.asarray(ln_gamma, dtype=np.float32)
    ln_beta = np.asarray(ln_beta, dtype=np.float32)
    w_qkv = np.asarray(w_qkv, dtype=np.float32)
    w_out = np.asarray(w_out, dtype=np.float32)

    wsc = w_qkv.copy()
    wsc[:, :INNER] *= SCALE                      # fold attn scale into q
    wfold = ln_gamma[:, None] * wsc              # fold LN gamma
    u = wfold.sum(axis=0)                        # [3*INNER]
    v0 = ln_beta @ wsc                           # [3*INNER]
    has_v0 = bool(np.any(v0 != 0.0))

    wq, wk, wv_all = np.split(wfold, 3, axis=1)
    uq, uk, uv_all = np.split(u, 3)
    v0q, v0k, v0v_all = np.split(v0, 3)

    in_maps = []
    for c in range(8):
        b = c // 4
        hs = (c % 4) * HL * DH
        sl = slice(hs, hs + HL * DH)
        xb = x[b]                                           # [2048, 1024]
        wqk_loc = np.concatenate([wq[:, sl], wk[:, sl]], axis=1)  # [1024, 512]
        in_maps.append({
            "xt": np.ascontiguousarray(xb.T).astype(_BF16),
            "xn": np.ascontiguousarray(xb).astype(_BF16),
            "wqk": np.ascontiguousarray(wqk_loc).astype(_BF16),
            "wv": np.ascontiguousarray(wv_all[:, sl]).astype(_BF16),
            "wout": np.ascontiguousarray(w_out[sl, :]).astype(_BF16),
            "uqk": np.concatenate([uq[sl], uk[sl]]).astype(np.float32),
            "uv": uv_all[sl].astype(np.float32),
            "v0qk": np.concatenate([v0q[sl], v0k[sl]]).astype(np.float32),
            "v0v": v0v_all[sl].astype(np.float32),
        })
    return in_maps, has_v0


def run(x, ln_gamma, ln_beta, w_qkv, w_out, b_out, trace=False, trace_kwargs=None):
    in_maps, has_v0 = _prep_inputs(x, ln_gamma, ln_beta, w_qkv, w_out, b_out)
    key = ("nc", has_v0)
    if key not in _CACHE:
        _CACHE[key] = _build(has_v0)
    nc = _CACHE[key]
    kwargs = {}
    if trace:
        kwargs = dict(trace=True, trace_cores=[0],
                      stitch_traces=False, **(trace_kwargs or {}))
    res = bass_utils.run_bass_kernel_spmd(
        nc, in_maps, core_ids=list(range(8)), **kwargs)

    b_out = np.asarray(b_out, dtype=np.float32)
    out = np.zeros((B, N, DIM), dtype=np.float32)
    for b in range(B):
        acc = np.zeros((DIM, T), dtype=np.float32)
        for c in range(4 * b, 4 * b + 4):
            acc += res.results[c]["outp0"]
            acc += res.results[c]["outp1"]
        out[b] = acc.T + b_out
    return out, res


def kernel(x, ln_gamma, ln_beta, w_qkv, w_out, b_out):
    out, _ = run(x, ln_gamma, ln_beta, w_qkv, w_out, b_out, trace=False)
    return out
